# revision 1
# baseline (speedup 1.0000x reference)
"""Sharded Bass kernel for nn_AggrGATGated: gated GNN message passing.

Sharding: nodes are sharded across the 8 cores. Each edge's gather index ==
its scatter index (the reference gathers src_gated[edge_idx] and scatter-adds
to the same edge_idx), so a core that owns a node range processes exactly the
edges targeting it: NO collectives are needed at all.

Within a core, nodes are re-packed into NB blocks of 128 PSUM slots by a
worst-fit-decreasing bin packer so that each (block, edge-type) holds <= 128
edges; this makes every (block, type) exactly one 128-edge matmul tile
(tile inflation ~1.03x vs the unpacked 1.53x). The host precomputes, per core:
transposed 128x128 edge-feature tiles (chunk-packed for >=512KB DMAs), per-edge
block offsets, and int16 gather indices (wrapped 16-partition layout,
replicated for the 8 GPSIMD cores).

Per-core device program:
  phase 1: sg = features_shard @ W_gate  -> bf16 table in DRAM (scatter target
           slots), written in JB-block groups.
  phase 2: per tile (block b, type t):
     pgv[:, 0:2U]  = efT.T @ [W_gate_e[t] | W_dense[t]]   (one bf16 matmul)
     pgv[:, 0:U]  += I.T @ sg_rows       (sg rows fetched with dma_gather)
     pgv[:, U:2U] += ones.T @ b[t]       (K=1 bias matmul)
     gate = sigmoid(pgv[:, 0:U])         (ACT)
     msgs = gate * pgv[:, U:2U]          (DVE, bf16 out)
     onehot = (iota == off)              (GPSIMD tensor_scalar; pads off=255)
     out_psum[b] += onehot.T @ msgs      (scatter by matmul, PSUM-accumulated;
                                          emitted LAG tiles late for pipelining)
  block psums are flushed to staging (DVE) and stored in OB-block DMA groups.

Matmul inputs are bf16 (PE full speed, psum accumulation fp32); output fp32.
"""
import dataclasses
import numpy as np
import ml_dtypes

def _bf(x):
    return np.asarray(x).astype(ml_dtypes.bfloat16)

import concourse.bass as bass
import concourse.bacc as bacc
import concourse.mybir as mybir
from concourse.tile import TileContext

F32 = mybir.dt.float32
F32R = mybir.dt.float32r
I32 = mybir.dt.int32
I16 = mybir.dt.int16
BF16 = mybir.dt.bfloat16
AF = mybir.ActivationFunctionType
ALU = mybir.AluOpType


@dataclasses.dataclass
class Cfg:
    ncores: int = 8
    R: int = 12544          # real node coverage per core (ceil(BN/8) to 128)
    NB: int = 132           # device blocks per core (>= R/128; slack for packing)
    F: int = 256            # node feature dim
    U: int = 256            # output dim
    FE: int = 128           # edge feature dim
    T: int = 3              # edge types
    BN: int = 100000        # real node count (B*N)
    GCH: int = 8            # gather/eft chunk, in tiles (1024 idx = SWDGE ring limit)
    JB: int = 8             # featT/sg blocks per DMA group
    OB: int = 4             # out blocks per DMA group

    @property
    def NBLK(self):
        return self.NB

    @property
    def RS(self):
        return self.NB * 128    # device node slots per core


def _pack_core(d: np.ndarray, NB: int, cap: int = 128):
    """Assign nodes (degree vectors d [Rn, T]) to NB blocks of <=128 slots with
    per-type edge-count <= cap. Worst-fit decreasing; overflows allowed (they
    just bump the tile count). Returns assign [Rn]."""
    Rn, T = d.shape
    order = np.argsort(-d.sum(axis=1), kind='stable')
    rem = np.full((NB, T), cap, np.int64)
    slots = np.full(NB, 128, np.int64)
    assign = np.empty(Rn, np.int64)
    for n in order:
        dn = d[n]
        fits = (rem >= dn).all(axis=1) & (slots > 0)
        if fits.any():
            score = (rem - dn).min(axis=1).astype(np.float64)
            score[~fits] = -np.inf
            b = int(np.argmax(score * 128 + slots))
        else:
            ok = slots > 0
            over = np.maximum(dn - rem, 0).sum(axis=1).astype(np.float64)
            over[~ok] = np.inf
            b = int(np.argmin(over))
        assign[n] = b
        rem[b] -= dn
        slots[b] -= 1
    return assign


def preprocess(cfg: Cfg, edge_idx: np.ndarray, edge_feats: np.ndarray):
    """Pack nodes into blocks, bucket edges per (block, type) tile.

    Returns (K, NT, per_core, slot_of_node) where per_core holds the device
    input arrays and slot_of_node [NC, R] maps local node -> device slot."""
    NC, R, NB, T, FE, GCH = cfg.ncores, cfg.R, cfg.NB, cfg.T, cfg.FE, cfg.GCH
    edge_idx = np.asarray(edge_idx)

    # per-node type degrees over the padded node space
    deg = np.zeros((NC * R, T), np.int32)
    for t in range(T):
        deg[:, t] = np.bincount(edge_idx[t], minlength=NC * R)[:NC * R]

    slot_of_node = np.zeros((NC, R), np.int64)
    for c in range(NC):
        assign = _pack_core(deg[c * R:(c + 1) * R], NB)
        order = np.argsort(assign, kind='stable')
        ranks = np.empty(R, np.int64)
        # rank within block
        blocksorted = assign[order]
        start = np.searchsorted(blocksorted, np.arange(NB))
        pos = np.arange(R) - start[blocksorted]
        ranks[order] = pos
        slot_of_node[c] = assign * 128 + ranks

    # per (core, block, type) counts using slots
    counts = np.zeros((NC, NB, T), np.int64)
    eslots = []          # per t: (sorted edge ids, their slots, core id)
    for t in range(T):
        idx = edge_idx[t]
        core = idx // R
        loc = idx - core * R
        slot = slot_of_node[core, loc]
        key = core * (NB * 128) + slot
        o = np.argsort(key, kind='stable')
        eslots.append((o, core[o], slot[o]))
        blk = core[o] * NB + (slot[o] >> 7)
        cnt = np.bincount(blk, minlength=NC * NB)
        counts[:, :, t] = cnt.reshape(NC, NB)

    K = -(-counts.max(axis=0) // 128)        # [NB, T], may contain 0
    NT = int(K.sum())
    NCH = -(-NT // GCH)
    Kcum = np.zeros((NB, T), np.int64)
    acc = 0
    for b in range(NB):
        for t in range(T):
            Kcum[b, t] = acc
            acc += int(K[b, t])

    per_core = []
    for c in range(NC):
        ids = np.full((NT, 128), -1, dtype=np.int64)
        offs = np.full((NT, 128), 255, dtype=np.float32)
        gidx = np.zeros((NT, 128), dtype=np.int16)
        for t in range(T):
            o, ecore, eslot = eslots[t]
            lo = np.searchsorted(ecore, c)
            hi = np.searchsorted(ecore, c + 1)
            sl = eslot[lo:hi]
            eid = o[lo:hi]
            bounds = np.searchsorted(sl, np.arange(NB + 1) * 128)
            for b in range(NB):
                s, e = bounds[b], bounds[b + 1]
                n = e - s
                if n == 0:
                    continue
                ti = int(Kcum[b, t])
                for k in range(int(K[b, t])):
                    a0, a1 = k * 128, min((k + 1) * 128, n)
                    m = a1 - a0
                    if m <= 0:
                        break
                    ids[ti + k, :m] = eid[s + a0:s + a1]
                    offs[ti + k, :m] = (sl[s + a0:s + a1] & 127).astype(np.float32)
                    gidx[ti + k, :m] = sl[s + a0:s + a1].astype(np.int16)
        # eft tiles (transposed), chunk-major packing
        valid = ids >= 0
        type_of_tile = np.zeros(NT, np.int64)
        for b in range(NB):
            for t in range(T):
                ti = int(Kcum[b, t])
                type_of_tile[ti:ti + int(K[b, t])] = t
        eft = np.zeros((NT, 128, FE), dtype=np.float32)
        for t in range(T):
            sel = np.nonzero(type_of_tile == t)[0]
            idsf = ids[sel]
            v = idsf >= 0
            ef = np.zeros((len(sel), 128, FE), np.float32)
            ef[v] = np.asarray(edge_feats[t])[idsf[v]]
            eft[sel] = ef
        eftT = eft.transpose(0, 2, 1)
        eftC = np.zeros((NCH, FE, GCH * 128), ml_dtypes.bfloat16)
        for ch in range(NCH):
            n_t = min(GCH, NT - ch * GCH)
            blk = eftT[ch * GCH: ch * GCH + n_t]
            eftC[ch, :, :n_t * 128] = blk.transpose(1, 0, 2).reshape(FE, n_t * 128)
        offsT = np.ascontiguousarray(offs.T).astype(np.float32)
        gflat = gidx.reshape(-1)
        gwrap = np.tile(gflat.reshape(NT * 8, 16).T, (8, 1))
        per_core.append(dict(eft=eftC, offsT=offsT, gidx=gwrap))
    return K, NT, per_core, slot_of_node


def make_feat_inputs(cfg: Cfg, features: np.ndarray, slot_of_node: np.ndarray):
    """Per-core packed featT over device slots: [NBJ, 128, JB*FKC*128]."""
    NC, R, F, JB, NB = cfg.ncores, cfg.R, cfg.F, cfg.JB, cfg.NB
    FKC = F // 128
    RS = cfg.RS
    NBJ = -(-NB // JB)
    feat_flat = np.asarray(features).reshape(-1, F)
    outs = []
    for c in range(NC):
        fs = np.zeros((RS, F), np.float32)
        lo, hi = c * R, min((c + 1) * R, feat_flat.shape[0])
        if hi > lo:
            fs[slot_of_node[c][:hi - lo]] = feat_flat[lo:hi]
        fc = fs.reshape(NB, 128, FKC, 128)
        ft = fc.transpose(0, 2, 3, 1)                # [NB, FKC, f, n]
        packed = np.zeros((NBJ, 128, JB * FKC * 128), ml_dtypes.bfloat16)
        for jc in range(NBJ):
            nb = min(JB, NB - jc * JB)
            blk = ft[jc * JB: jc * JB + nb]
            packed[jc, :, :nb * FKC * 128] = (
                blk.transpose(2, 0, 1, 3).reshape(128, nb * FKC * 128))
        outs.append(packed)
    return outs


def build_kernel(cfg: Cfg, K: np.ndarray, NT: int, dbg: bool = False, bench_iters: int = 0, ablate: str = ''):
    NBLK, T, U, FE, F = cfg.NBLK, cfg.T, cfg.U, cfg.FE, cfg.F
    GCH, JB, OB = cfg.GCH, cfg.JB, cfg.OB
    FKC = F // 128
    NCH = -(-NT // GCH)
    NBJ = -(-NBLK // JB)

    nc = bacc.Bacc("TRN2", target_bir_lowering=False, debug=False,
                   num_devices=cfg.ncores)

    featT = nc.dram_tensor("featT", [NBJ, 128, JB * FKC * 128], BF16,
                           kind="ExternalInput")
    wg = nc.dram_tensor("wg", [FKC, 128, U], BF16, kind="ExternalInput")
    wcat = nc.dram_tensor("wcat", [T, FE, 2 * U], BF16, kind="ExternalInput")
    bvec = nc.dram_tensor("bvec", [T, 1, U], BF16, kind="ExternalInput")
    ones = nc.dram_tensor("ones", [1, 128], BF16, kind="ExternalInput")
    iota = nc.dram_tensor("iota", [128, 128], F32, kind="ExternalInput")
    ident = nc.dram_tensor("ident", [128, 128], BF16, kind="ExternalInput")
    eft = nc.dram_tensor("eft", [NCH, FE, GCH * 128], BF16, kind="ExternalInput")
    offsT = nc.dram_tensor("offsT", [128, NT], F32, kind="ExternalInput")
    out = nc.dram_tensor("out", [NBLK, 128, U], F32, kind="ExternalOutput")

    with TileContext(nc) as tc:
        with (
            tc.tile_pool(name="const", bufs=1) as constp,
            tc.tile_pool(name="ftile", bufs=3) as ftp,
            tc.tile_pool(name="eftl", bufs=4) as eftp,
            tc.tile_pool(name="oh2", bufs=6) as oh2p,
            tc.tile_pool(name="gate", bufs=8) as gatep,
            tc.tile_pool(name="msgs", bufs=8) as msgsp,
            tc.tile_pool(name="oh", bufs=14) as ohp,
            tc.tile_pool(name="outst", bufs=2) as outstp,
            tc.tile_pool(name="psg", bufs=2, space="PSUM") as psgp,
            tc.tile_pool(name="psv", bufs=2, space="PSUM") as psvp,
            tc.tile_pool(name="psout", bufs=2, space="PSUM") as psoutp,
            tc.tile_pool(name="pht", bufs=2, space="PSUM") as phtp,
        ):
            # ---- constants ----
            wg_sb = []
            for kc in range(FKC):
                w = constp.tile([128, U], BF16, tag=f"wg{kc}")
                nc.sync.dma_start(out=w[:, :], in_=wg[kc, :, :])
                wg_sb.append(w)
            wcat_sb, b_sb = [], []
            for t in range(T):
                w = constp.tile([FE, 2 * U], BF16, tag=f"wcat{t}")
                nc.scalar.dma_start(out=w[:, :], in_=wcat[t, :, :])
                wcat_sb.append(w)
                w = constp.tile([1, U], BF16, tag=f"b{t}")
                nc.scalar.dma_start(out=w[:, :], in_=bvec[t, :, :])
                b_sb.append(w)
            # non-urgent constants are loaded on the scalar HWDGE queue so
            # they don't delay the phase-1 featT stream on the sync queue
            ones_sb = constp.tile([1, 128], BF16, tag="ones")
            nc.scalar.dma_start(out=ones_sb[:, :], in_=ones[:, :])
            iota_sb = constp.tile([128, 128], F32, tag="iota")
            nc.scalar.dma_start(out=iota_sb[:, :], in_=iota[:, :])
            id_sb = constp.tile([128, 128], BF16, tag="ident")
            nc.scalar.dma_start(out=id_sb[:, :], in_=ident[:, :])
            offsT_sb = constp.tile([128, NT], F32, tag="offsT")
            nc.scalar.dma_start(out=offsT_sb[:, :], in_=offsT[:, :])
            sgtab = constp.tile([128, NBLK * U], BF16, tag="sgtab")

            # ---- phase 1: sg table, SBUF-resident ----
            import contextlib
            loop_cm = (tc.For_i(0, bench_iters, 1, hint_engines=(
                mybir.EngineType.PE, mybir.EngineType.DVE,
                mybir.EngineType.Activation, mybir.EngineType.Pool,
                mybir.EngineType.SP))
                if bench_iters else contextlib.nullcontext())
            loop_ctx = loop_cm.__enter__() if bench_iters else None
            for jc in range(NBJ):
                nb = min(JB, NBLK - jc * JB)
                ft = ftp.tile([128, JB * FKC * 128], BF16)
                nc.sync.dma_start(out=ft[:, :nb * FKC * 128],
                                  in_=featT[jc, :, :nb * FKC * 128])
                for jj in range(nb):
                    j = jc * JB + jj
                    ps = psoutp.tile([128, U], F32, tag="pso")
                    for kc in range(FKC):
                        o = (jj * FKC + kc) * 128
                        nc.tensor.matmul(ps[:, :], ft[:, o:o + 128], wg_sb[kc][:, :],
                                         start=(kc == 0), stop=(kc == FKC - 1))
                    if jj % 2 == 0:
                        nc.scalar.copy(sgtab[:, j * U:(j + 1) * U], ps[:, :])
                    else:
                        nc.vector.tensor_copy(sgtab[:, j * U:(j + 1) * U], ps[:, :])

            # ---- phase 2 ----
            eft_tiles = {}
            if 'phase1' in ablate:
                NBLK_eff = 0
            else:
                NBLK_eff = NBLK

            def ensure_chunk(g):
                if g in eft_tiles:
                    return
                t0 = g * GCH
                n_t = min(GCH, NT - t0)
                et = eftp.tile([FE, GCH * 128], BF16, tag="et", name=f"et{g}")
                nc.sync.dma_start(out=et[:, :n_t * 128],
                                  in_=eft[g, :, :n_t * 128])
                eft_tiles[g] = et

            ost = None
            LAG = 10
            pending = []          # (oh, msgs_ap, pso, start, stop, flush_or_None)
            state = dict(ost=None)

            def emit_scatter(ent):
                oh_, msgs_ap, pso_, st_, sp_, flush = ent
                nc.tensor.matmul(pso_[:, :], oh_[:, :], msgs_ap,
                                 start=st_, stop=sp_)
                if flush is not None:
                    flush()

            # flat tile schedule: (tile index, block, type, first/last in block)
            sched = []
            for b in range(NBLK_eff):
                ntile_b = int(K[b].sum())
                done = 0
                for t in range(T):
                    for k in range(int(K[b, t])):
                        sched.append((b, t, done == 0, done == ntile_b - 1))
                        done += 1
            psos = {}
            flushes_due = {}

            def block_prolog(b):
                ntile_b = int(K[b].sum())
                psos[b] = psoutp.tile([128, U], F32, tag="pso", name=f"pso{b}")
                return psos[b]

            ntiles_of = [int(K[b].sum()) for b in range(NBLK)]
            dma_owner = {}
            for g0 in range(0, NBLK, OB):
                grp = [b for b in range(g0, min(g0 + OB, NBLK))]
                live = [b for b in grp if ntiles_of[b] > 0]
                dma_owner[g0] = live[-1] if live else None

            def emit_group_store(g0):
                nb = min(OB, NBLK - g0)
                ost = state['ost']
                nc.sync.dma_start(
                    out=out[g0:g0 + nb, :, :].rearrange("j p u -> p j u"),
                    in_=ost[:, :nb * U].rearrange("p (j u) -> p j u", u=U))

            def block_epilog(b):
                """Returns the flush closure for non-empty block b."""
                bo = b % OB
                pso = psos.get(b)
                g0 = (b // OB) * OB

                def flush(b=b, bo=bo, ost=state['ost'], pso=pso, g0=g0):
                    nc.scalar.copy(ost[:, bo * U:(bo + 1) * U], pso[:, :])
                    if dma_owner[g0] == b:
                        nb = min(OB, NBLK - g0)
                        nc.sync.dma_start(
                            out=out[g0:g0 + nb, :, :]
                                .rearrange("j p u -> p j u"),
                            in_=ost[:, :nb * U]
                                .rearrange("p (j u) -> p j u", u=U))
                return flush

            def ost_prolog(b):
                g0 = (b // OB) * OB
                if state.get('ost_g0') == g0:
                    return
                state['ost_g0'] = g0
                state['ost'] = outstp.tile([128, OB * U], F32,
                                           tag="ost", name=f"ost{g0}")
                grp = range(g0, min(g0 + OB, NBLK))
                if any(ntiles_of[bb] == 0 for bb in grp):
                    nc.vector.memset(state['ost'][:, :], 0.0)
                    if dma_owner[g0] is None:
                        emit_group_store(g0)

            # emit in pairs: shared [gateA|gateB] / [valA|valB] psum banks so
            # sigmoid and the gate*val multiply run once per pair at [128,512]
            NTs = len(sched)
            i = 0
            while i < NTs:
                npair = 2 if i + 1 < NTs else 1
                idxs = list(range(i, i + npair))
                for ii in idxs:
                    g = ii // GCH
                    ensure_chunk(g)
                    if ii % GCH == 0:
                        for gg in (g + 1, g + 2):
                            if gg * GCH < NT:
                                ensure_chunk(gg)
                ohs = []
                pht = phtp.tile([128, 2 * 128], BF16, tag="oht",
                                name=f"pht{i}")
                for h, ii in enumerate(idxs):
                    oh = ohp.tile([128, 128], BF16, tag="oh", name=f"oh{ii}")
                    nc.vector.tensor_scalar(oh[:, :], iota_sb[:, :],
                                            offsT_sb[:, ii:ii + 1], None,
                                            ALU.is_equal)
                    nc.tensor.transpose(pht[:, h * 128:(h + 1) * 128],
                                        oh[:, :], id_sb[:, :])
                    ohs.append(oh)
                oh2 = oh2p.tile([128, 2 * 128], BF16)
                if (i // 2) % 2 == 0:
                    nc.scalar.copy(oh2[:, :npair * 128], pht[:, :npair * 128])
                else:
                    nc.vector.tensor_copy(oh2[:, :npair * 128],
                                          pht[:, :npair * 128])
                bankG = psgp.tile([128, 2 * U], F32, tag="gg")
                bankV = psvp.tile([128, 2 * U], F32, tag="vv")
                for h, ii in enumerate(idxs):
                    b, t, first, last = sched[ii]
                    g, s = divmod(ii, GCH)
                    ef = eft_tiles[g][:, s * 128:(s + 1) * 128]
                    hU = h * U
                    nc.tensor.matmul(bankG[:, hU:hU + U], ef,
                                     wcat_sb[t][:, 0:U],
                                     start=True, stop=False,
                                     skip_group_check=True)
                    if 'nogather' not in ablate:
                        nc.tensor.matmul(bankG[:, hU:hU + U],
                                         oh2[:, h * 128:(h + 1) * 128],
                                         sgtab[:, b * U:(b + 1) * U],
                                         start=False, stop=True,
                                         skip_group_check=True)
                    else:
                        nc.tensor.matmul(bankG[:, hU:hU + U], ones_sb[:, :],
                                         b_sb[t][:, :], start=False, stop=True,
                                         skip_group_check=True)
                    nc.tensor.matmul(bankV[:, hU:hU + U], ef,
                                     wcat_sb[t][:, U:2 * U],
                                     start=True, stop=False,
                                     skip_group_check=True)
                    nc.tensor.matmul(bankV[:, hU:hU + U], ones_sb[:, :],
                                     b_sb[t][:, :], start=False, stop=True,
                                     skip_group_check=True)
                gate = gatep.tile([128, 2 * U], F32)
                nc.scalar.activation(gate[:, :npair * U],
                                     bankG[:, :npair * U], AF.Sigmoid)
                msgs = msgsp.tile([128, 2 * U], BF16)
                nc.vector.tensor_tensor(msgs[:, :npair * U],
                                        gate[:, :npair * U],
                                        bankV[:, :npair * U], ALU.mult)
                for h, ii in enumerate(idxs):
                    b, t, first, last = sched[ii]
                    if first:
                        ost_prolog(b)
                        block_prolog(b)
                    if 'noscatter' not in ablate:
                        pending.append([ohs[h], msgs[:, h * U:(h + 1) * U],
                                        psos[b], first, last, None])
                        if last:
                            pending[-1][5] = block_epilog(b)
                        if len(pending) > LAG:
                            emit_scatter(pending.pop(0))
                i += npair
            for ent in pending:
                emit_scatter(ent)
            # groups consisting entirely of empty blocks
            if NBLK_eff:
                for g0 in range(0, NBLK, OB):
                    if dma_owner[g0] is None and state.get('ost_g0') != g0:
                        ost_prolog(g0)
            if bench_iters:
                loop_cm.__exit__(None, None, None)
    nc.compile()
    return nc


def make_const_inputs(cfg: Cfg, W_gate, W_gate_e, W_dense, b_dense):
    FKC = cfg.F // 128
    return dict(
        wg=_bf(np.ascontiguousarray(
            np.asarray(W_gate, np.float32).reshape(FKC, 128, cfg.U))),
        wcat=_bf(np.concatenate([np.asarray(W_gate_e, np.float32),
                                 np.asarray(W_dense, np.float32)], axis=2)),
        bvec=_bf(np.asarray(b_dense, np.float32).reshape(cfg.T, 1, cfg.U)),
        ones=np.ones((1, 128), ml_dtypes.bfloat16),
        iota=np.broadcast_to(np.arange(128, dtype=np.float32), (128, 128)).copy(),
        ident=np.eye(128).astype(ml_dtypes.bfloat16),
    )


def make_in_maps(cfg: Cfg, inputs):
    K, NT, per_core, slot_of_node = preprocess(
        cfg, inputs['edge_idx'], inputs['edge_feats'])
    feat_in = make_feat_inputs(cfg, inputs['features'], slot_of_node)
    const_in = make_const_inputs(cfg, inputs['W_gate'], inputs['W_gate_e'],
                                 inputs['W_dense'], inputs['b_dense'])
    in_maps = []
    for c in range(cfg.ncores):
        m = dict(const_in)
        m['featT'] = feat_in[c]
        m.update(per_core[c])
        in_maps.append(m)
    return K, NT, in_maps, slot_of_node


def extract_output(cfg: Cfg, results, slot_of_node):
    out_full = np.zeros((cfg.ncores * cfg.R, cfg.U), np.float32)
    for c in range(cfg.ncores):
        dev = results[c]['out'].reshape(cfg.RS, cfg.U)
        out_full[c * cfg.R:(c + 1) * cfg.R] = dev[slot_of_node[c]]
    return out_full[:cfg.BN]


def run_full(cfg: Cfg, inputs, run_fn):
    K, NT, in_maps, slot_of_node = make_in_maps(cfg, inputs)
    nc = build_kernel(cfg, K, NT)
    results = run_fn(nc, in_maps)
    return extract_output(cfg, results, slot_of_node)


# ============================================================================
# Self-contained entry point (harness contract):
#   kernel(**inputs) takes the FULL unsharded inputs and returns the FULL
#   output [2, 50000, 256] float32. Internally: node-shard across the 8
#   NeuronCores (no collectives needed since gather idx == scatter idx per
#   edge), compile one SPMD Bass program, run via run_bass_kernel_spmd.
# ============================================================================
from concourse.bass_utils import run_bass_kernel_spmd

_CACHE = {}


def kernel(features, edge_idx, edge_feats, W_gate, W_gate_e, W_dense, b_dense):
    features = np.asarray(features)
    edge_idx = np.asarray(edge_idx)
    edge_feats = np.asarray(edge_feats)
    B, N, F = features.shape
    BN = B * N
    cfg = Cfg(ncores=8, R=-(-BN // (8 * 128)) * 128, F=F,
              U=np.asarray(W_gate).shape[1], FE=edge_feats.shape[2],
              T=edge_feats.shape[0], BN=BN)
    cfg.NB = -(-cfg.R // 128) + 34      # packing slack (~35% spare slots)

    inputs = dict(features=features, edge_idx=edge_idx, edge_feats=edge_feats,
                  W_gate=W_gate, W_gate_e=W_gate_e, W_dense=W_dense,
                  b_dense=b_dense)
    K, NT, in_maps, slot_of_node = make_in_maps(cfg, inputs)

    key = (cfg.R, cfg.NB, cfg.F, cfg.U, cfg.FE, cfg.T, NT, K.tobytes())
    nc = _CACHE.get(key)
    if nc is None:
        nc = build_kernel(cfg, K, NT)
        _CACHE[key] = nc

    res = run_bass_kernel_spmd(nc, in_maps, core_ids=list(range(cfg.ncores)))
    out = extract_output(cfg, res.results, slot_of_node)
    return out.reshape(B, N, cfg.U).astype(np.float32)

